# revision 7
# baseline (speedup 1.0000x reference)
"""Trainium2 Bass kernel for nn_Contrastive_Loss (bs=8192, hidden=2048, 8 cores).

Math: reference(X, Y) = cl(X,Y) + cl(Y,X) where
  cl(A,B)[i] = -log(E_ii / (colsum_i(E) - E_ii)),  E = exp(norm(A) @ norm(B).T)
Since norm(Y)@norm(X).T = S.T, the second term's column sums are the first
term's row sums and the diagonals coincide.  With E = exp(S):
  out[i] = log(rowsum_i(E) - E_ii) + log(colsum_i(E) - E_ii) - 2*S_ii

Sharding (v2, transposed blocks): core r holds X rows [1024r, 1024(r+1))
and computes the transposed block E^T[all 8192 Y rows, its 1024 X rows].
Host supplies raw X^T shard, raw X/Y row-major shards, and the full raw
Y^T (bf16) - pure layout staging, no math.  On device:
 - X row norms via squared-X^T ones-matmul (row layout), 1/norm broadcast
   to all partitions via a rank-1 matmul, folded into X^T (the moving
   matmul operand) once up front,
 - Y row norms for the local shard on ACT; the 8192 1/||y_j|| values are
   the ONLY data AllGathered (32KB, vs 32MB of normalized Y in v1), and
   they fold into the exp activation's per-partition scale,
 - PE streams S^T tiles [128 j, 1024 i] (K=2048); ACT applies
   exp(raw * invy_j) and its free-dim accumulation yields the column-sum
   partials (the cross-device reduction) directly,
 - row sums via ones-matmul over E^T tiles (all-local, no collective),
 - ReduceScatter sums the 8192 column partials (32KB) so each core gets
   the 1024 column sums matching its own rows.
No transpose DMAs and no bulk collective remain; PE matmul (~470us of
streaming) is the critical path, matching target_regime=compute.
"""

import numpy as np
import ml_dtypes
from contextlib import ExitStack

import concourse.bass as bass
import concourse.bacc as bacc
import concourse.mybir as mybir
import concourse.tile as tile
from concourse.bass_utils import run_bass_kernel_spmd

FP32 = mybir.dt.float32
BF16 = mybir.dt.bfloat16

BS = 8192      # batch (rows of X and Y)
H = 2048       # hidden
NCORES = 8
RPC = BS // NCORES   # rows per core = 1024
EPS = 1e-8


def build(bs=BS, h=H, ncores=NCORES):
    rpc = bs // ncores
    kt_n = h // 128          # contraction tiles = 16
    jt_n = bs // 128         # j (Y-row) tiles per core = 64
    mb_n = rpc // 128        # own-row 128-blocks = 8
    gs = 8                   # j-tiles per YT load group
    ng = jt_n // gs          # load groups = 8
    groups = [list(range(ncores))]

    nc = bacc.Bacc("TRN2", target_bir_lowering=False, num_devices=ncores)
    XT = nc.dram_tensor("XT", [h, rpc], BF16, kind="ExternalInput")
    XS = nc.dram_tensor("XS", [rpc, h], BF16, kind="ExternalInput")
    YS = nc.dram_tensor("YS", [rpc, h], BF16, kind="ExternalInput")
    YT = nc.dram_tensor("YT", [h, bs], BF16, kind="ExternalInput")
    OUT = nc.dram_tensor("OUT", [rpc, 1], FP32, kind="ExternalOutput")

    with tile.TileContext(nc) as tc, ExitStack() as ctx:
        dram = ctx.enter_context(tc.tile_pool(name="dram", bufs=1, space="DRAM"))
        prep = ctx.enter_context(tc.tile_pool(name="prep", bufs=2))
        junkp = ctx.enter_context(tc.tile_pool(name="junkp", bufs=2))
        prodp = ctx.enter_context(tc.tile_pool(name="prodp", bufs=2))
        small = ctx.enter_context(tc.tile_pool(name="small", bufs=1))
        stat = ctx.enter_context(tc.tile_pool(name="stat", bufs=1))
        xpool = ctx.enter_context(tc.tile_pool(name="xpool", bufs=1))
        xsqp = ctx.enter_context(tc.tile_pool(name="xsqp", bufs=2))
        ypool = ctx.enter_context(tc.tile_pool(name="ypool", bufs=2 * kt_n))
        epool = ctx.enter_context(tc.tile_pool(name="epool", bufs=4))
        stpsum = ctx.enter_context(tc.tile_pool(name="stpsum", bufs=2, space="PSUM"))
        rpsum = ctx.enter_context(tc.tile_pool(name="rpsum", bufs=1, space="PSUM"))

        INVYL = dram.tile([rpc], FP32, name="INVYL", tag="INVYL")
        INVYA = dram.tile([bs], FP32, addr_space="Shared", name="INVYA", tag="INVYA")
        INVXD = dram.tile([rpc], FP32, name="INVXD", tag="INVXD")
        CS = dram.tile([bs], FP32, name="CSD", tag="CSD")
        CSR = dram.tile([rpc], FP32, name="CSRD", tag="CSRD")
        RSD = dram.tile([rpc], FP32, name="RSD", tag="RSD")

        # persistent stats (p = row % 128, column m = row // 128)
        ssqy = stat.tile([128, mb_n], FP32)        # ||y_p,m||^2 (own shard)
        sdraw = stat.tile([128, mb_n], FP32)       # raw x.y dot (own rows)
        invy_all = stat.tile([128, jt_n], FP32)    # 1/||y_j|| all 8192 rows
        csacc = stat.tile([128, jt_n, 2], FP32)    # colsum partials per (jt, half)
        ones = stat.tile([128, 1], BF16)
        ones_row = stat.tile([1, 128], BF16)
        invx_row = stat.tile([1, rpc], FP32)       # 1/||x_i|| in row layout
        invxb_bf = stat.tile([1, rpc], BF16)
        invxb = stat.tile([128, rpc], FP32)        # broadcast to all partitions

        nc.vector.memset(ones[:], 1.0)
        nc.vector.memset(ones_row[:], 1.0)

        # ---------------- Phase A1: Y shard norms + raw diagonal dots ------
        for m in range(mb_n):
            r0 = m * 128
            ys = prep.tile([128, h], BF16, tag="ldy", name="ldy")
            nc.gpsimd.dma_start(out=ys[:], in_=YS[r0 : r0 + 128, :])
            junk = junkp.tile([128, h], BF16, tag="junk", name="junk")
            nc.scalar.activation(
                junk[:], ys[:], mybir.ActivationFunctionType.Square,
                accum_out=ssqy[:, m : m + 1],
            )
            xs = prep.tile([128, h], BF16, tag="ldx", name="ldx")
            nc.gpsimd.dma_start(out=xs[:], in_=XS[r0 : r0 + 128, :])
            prod = prodp.tile([128, h], FP32, tag="prod", name="prod")
            nc.vector.tensor_mul(prod[:], xs[:], ys[:])
            nc.vector.reduce_sum(
                sdraw[:, m : m + 1], prod[:], axis=mybir.AxisListType.X
            )

        nrmy = small.tile([128, mb_n], FP32, tag="nrmy", name="nrmy")
        nc.scalar.sqrt(nrmy[:], ssqy[:])
        nc.vector.tensor_scalar_max(nrmy[:], nrmy[:], EPS)
        invy_own = stat.tile([128, mb_n], FP32)
        nc.vector.reciprocal(invy_own[:], nrmy[:])

        # AllGather the 8192 1/||y|| values (32KB total)
        nc.gpsimd.dma_start(
            out=INVYL.rearrange("(a b) -> b a", b=128), in_=invy_own[:]
        )
        nc.gpsimd.collective_compute(
            "AllGather", mybir.AluOpType.bypass, replica_groups=groups,
            ins=[INVYL.opt()], outs=[INVYA.opt()],
        )
        nc.gpsimd.dma_start(
            out=invy_all[:], in_=INVYA.rearrange("(a b) -> b a", b=128)
        )

        # ---------------- Phase A2: X^T load, norms, fold 1/||x|| ----------
        # xnt[p, k, i] = X[shard_row i, 128k+p]
        xnt = xpool.tile([128, kt_n, rpc], BF16)
        nc.gpsimd.dma_start(out=xnt[:], in_=XT.rearrange("(k p) m -> p k m", p=128))

        # sum_k X[i,k]^2 via ones-matmul on squared X^T -> row layout [1, rpc]
        xss = [rpsum.tile([1, 512], FP32, tag=f"xss{i}", name=f"xss{i}")
               for i in range(2)]
        for kt in range(kt_n):
            xsq = xsqp.tile([128, rpc], BF16, tag="xsq", name="xsq")
            nc.scalar.activation(
                xsq[:], xnt[:, kt, :], mybir.ActivationFunctionType.Square
            )
            for hh in range(2):
                nc.tensor.matmul(
                    xss[hh][:], lhsT=ones[:], rhs=xsq[:, hh * 512 : hh * 512 + 512],
                    start=(kt == 0), stop=(kt == kt_n - 1),
                )
        xsr = small.tile([1, rpc], FP32, tag="xsr", name="xsr")
        for hh in range(2):
            nc.vector.tensor_copy(xsr[:, hh * 512 : hh * 512 + 512], xss[hh][:])
        nrmx = small.tile([1, rpc], FP32, tag="nrmx", name="nrmx")
        nc.scalar.sqrt(nrmx[:], xsr[:])
        nc.vector.tensor_scalar_max(nrmx[:], nrmx[:], EPS)
        nc.vector.reciprocal(invx_row[:], nrmx[:])
        nc.vector.tensor_copy(invxb_bf[:], invx_row[:])

        # broadcast 1/||x|| to all 128 partitions via rank-1 matmul
        bps = stpsum.tile([128, rpc], FP32, tag="st", name="bps")
        for hh in range(2):
            nc.tensor.matmul(
                bps[:, hh * 512 : hh * 512 + 512], lhsT=ones_row[:],
                rhs=invxb_bf[:, hh * 512 : hh * 512 + 512],
                start=True, stop=True,
            )
        nc.vector.tensor_copy(invxb[:], bps[:])

        # fold into X^T in place: xnt[:, kt, i] *= invx[i]
        for kt in range(kt_n):
            nc.vector.tensor_mul(xnt[:, kt, :], xnt[:, kt, :], invxb[:])

        # diagonal: S_ii = (x_i . y_i) / (||x_i|| ||y_i||)
        invx_own = stat.tile([128, mb_n], FP32)
        nc.gpsimd.dma_start(out=INVXD[:], in_=invx_row[:])
        nc.gpsimd.dma_start(
            out=invx_own[:], in_=INVXD.rearrange("(a b) -> b a", b=128)
        )
        sdiag = stat.tile([128, mb_n], FP32)
        nc.vector.tensor_mul(sdiag[:], sdraw[:], invx_own[:])
        nc.vector.tensor_mul(sdiag[:], sdiag[:], invy_own[:])

        # ---------------- Phase B: S^T blocks, exp, row/col sums -----------
        rowps = [rpsum.tile([1, 512], FP32, tag=f"rps{i}", name=f"rps{i}")
                 for i in range(2)]

        def flush_ones(prev):
            pjt, pe = prev
            for hh in range(2):
                nc.tensor.matmul(
                    rowps[hh][:], lhsT=ones[:],
                    rhs=pe[:, hh * 512 : hh * 512 + 512],
                    start=(pjt == 0), stop=(pjt == jt_n - 1),
                )

        prev = None
        for g in range(ng):
            yts = []
            for kt in range(kt_n):
                yt_t = ypool.tile([128, 1024], BF16, tag="yt", name=f"yt{kt}")
                nc.sync.dma_start(
                    out=yt_t[:],
                    in_=YT[kt * 128 : kt * 128 + 128, g * 1024 : g * 1024 + 1024],
                )
                yts.append(yt_t)
            for jl in range(gs):
                jt = g * gs + jl
                st = stpsum.tile([128, 1024], FP32, tag="st", name="st")
                for hh in range(2):
                    for kt in range(kt_n):
                        nc.tensor.matmul(
                            st[:, hh * 512 : hh * 512 + 512],
                            lhsT=yts[kt][:, jl * 128 : jl * 128 + 128],
                            rhs=xnt[:, kt, hh * 512 : hh * 512 + 512],
                            start=(kt == 0), stop=(kt == kt_n - 1),
                        )
                if prev is not None:
                    flush_ones(prev)
                e = epool.tile([128, 1024], BF16, tag="E", name="E")
                for hh in range(2):
                    nc.scalar.activation(
                        e[:, hh * 512 : hh * 512 + 512],
                        st[:, hh * 512 : hh * 512 + 512],
                        mybir.ActivationFunctionType.Exp,
                        scale=invy_all[:, jt : jt + 1],
                        accum_out=csacc[:, jt, hh : hh + 1],
                    )
                prev = (jt, e)
        flush_ones(prev)

        # ---------------- ReduceScatter column sums ----------------
        cstot = stat.tile([128, jt_n], FP32)
        nc.vector.reduce_sum(cstot[:], csacc[:], axis=mybir.AxisListType.X)
        nc.gpsimd.dma_start(
            out=CS.rearrange("(a b) -> b a", b=128), in_=cstot[:]
        )
        nc.gpsimd.collective_compute(
            "ReduceScatter", mybir.AluOpType.add, replica_groups=groups,
            ins=[CS.opt()], outs=[CSR.opt()],
        )

        # ---------------- Finale ----------------
        csr = stat.tile([128, mb_n], FP32)
        nc.gpsimd.dma_start(out=csr[:], in_=CSR.rearrange("(a b) -> b a", b=128))
        rs_row = stat.tile([1, rpc], FP32)
        for hh in range(2):
            nc.vector.tensor_copy(rs_row[:, hh * 512 : hh * 512 + 512], rowps[hh][:])
        nc.gpsimd.dma_start(out=RSD[:], in_=rs_row[:])
        rsum = stat.tile([128, mb_n], FP32)
        nc.gpsimd.dma_start(out=rsum[:], in_=RSD.rearrange("(a b) -> b a", b=128))

        edig = stat.tile([128, mb_n], FP32)
        nc.scalar.activation(edig[:], sdiag[:], mybir.ActivationFunctionType.Exp)
        negr = stat.tile([128, mb_n], FP32)
        nc.vector.tensor_sub(negr[:], rsum[:], edig[:])
        negc = stat.tile([128, mb_n], FP32)
        nc.vector.tensor_sub(negc[:], csr[:], edig[:])
        lr = stat.tile([128, mb_n], FP32)
        nc.scalar.activation(lr[:], negr[:], mybir.ActivationFunctionType.Ln)
        lcv = stat.tile([128, mb_n], FP32)
        nc.scalar.activation(lcv[:], negc[:], mybir.ActivationFunctionType.Ln)
        res = stat.tile([128, mb_n], FP32)
        nc.vector.tensor_add(res[:], lr[:], lcv[:])
        d2 = stat.tile([128, mb_n], FP32)
        nc.vector.tensor_scalar_mul(d2[:], sdiag[:], -2.0)
        nc.vector.tensor_add(res[:], res[:], d2[:])
        nc.gpsimd.dma_start(
            out=OUT.rearrange("(a b) c -> b (a c)", b=128), in_=res[:]
        )

    nc.compile()
    return nc


_CACHE = {}


def _get_nc():
    if "nc" not in _CACHE:
        _CACHE["nc"] = build()
    return _CACHE["nc"]


def make_in_maps(X, Y, ncores=NCORES, rpc=RPC):
    bf16 = ml_dtypes.bfloat16
    yt = np.ascontiguousarray(Y.T.astype(bf16))
    maps = []
    for i in range(ncores):
        xs = X[i * rpc : (i + 1) * rpc]
        maps.append({
            "XT": np.ascontiguousarray(xs.T.astype(bf16)),
            "XS": np.ascontiguousarray(xs.astype(bf16)),
            "YS": np.ascontiguousarray(Y[i * rpc : (i + 1) * rpc].astype(bf16)),
            "YT": yt,
        })
    return maps


def kernel(X, Y):
    X = np.ascontiguousarray(np.asarray(X, dtype=np.float32))
    Y = np.ascontiguousarray(np.asarray(Y, dtype=np.float32))
    assert X.shape == (BS, H) and Y.shape == (BS, H)
    nc = _get_nc()
    r = run_bass_kernel_spmd(nc, make_in_maps(X, Y), list(range(NCORES)))
    out = np.concatenate([r.results[i]["OUT"] for i in range(NCORES)], axis=0)
    return out.astype(np.float32)
